# revision 3
# baseline (speedup 1.0000x reference)
"""Self-contained Trainium2 Bass kernel for the 2-layer SuperGAT-MX GNN
(nn_Net_1846835938183).

Strategy (edge/graph-partition parallelism over 8 NeuronCores):
  * Nodes are sorted by in-degree and grouped into "supertile groups" of
    n_cores*SN nodes; core c owns the c-th SN-node chunk of every group, so one
    static SPMD program (one NEFF) serves all 8 cores; per-core data arrives
    via inputs (permuted x block + per-edge gather indices).
  * Per layer, each core builds its block of an augmented node table
    T = [h | h@A_l | h@A_r] with PE matmuls (A_l/A_r fold the per-head att
    vectors into small matrices, precomputed on host), then the blocks are
    AllGathered so every core holds the full table in its HBM.
  * Per-edge work: src rows of T are fetched with indirect DMA gathers (one
    row per partition per descriptor set; 128 rows/instruction), the
    GO-gated attention (logits dot, sigmoid gate, leaky-relu, per-target
    segment softmax over the padded degree axis, weighted aggregation) runs on
    the Vector/Scalar engines with strided access-pattern views; everything is
    dense because edges are laid out node-per-partition x padded-degree.
  * Padded edge slots point at a dedicated GHOST table row (h=0, al=-1e30):
    alpha becomes -1e29 so exp(alpha - amax) == 0 exactly — masking costs
    nothing.
  * Layer-2 repeats the same edge stage on T2 (built fused with layer 1's
    output: elu -> PE transpose -> matmul), then mean-over-heads + bias +
    log_softmax, and each core writes its contiguous output block; the host
    inverse-permutes rows to the original node order.
"""
import sys
for _p in ("/opt/trn_rl_repo",):
    if _p not in sys.path:
        sys.path.insert(0, _p)

import numpy as np

N_NODES = 100000
F_IN = 256
HEADS = 8
C1 = 8
NCLS = 7
N_CORES = 8
SN = 256          # nodes per supertile per core
P = 128
RU = 4
NEG_SLOPE = 0.2

_nc_cache = {}


# ---------------------------------------------------------------- host prep --
def _head_expand(att, heads, ch):
    a = np.asarray(att, np.float32).reshape(1, heads, ch)
    m = np.zeros((heads * ch, heads), np.float32)
    for h in range(heads):
        m[h * ch:(h + 1) * ch, h] = a[0, h, :]
    return m


def _preprocess(edge_index):
    src0 = np.asarray(edge_index[0], np.int64)
    dst0 = np.asarray(edge_index[1], np.int64)
    loop = np.arange(N_NODES, dtype=np.int64)
    src = np.concatenate([src0, loop])
    dst = np.concatenate([dst0, loop])
    e2 = src.shape[0]

    deg = np.bincount(dst, minlength=N_NODES)
    node_order = np.argsort(-deg, kind="stable")
    invorder = np.empty(N_NODES, np.int64)
    invorder[node_order] = np.arange(N_NODES)

    grp = N_CORES * SN
    n_super = -(-N_NODES // grp)
    b_rows = n_super * SN
    n_rows = N_CORES * b_rows
    ghost_row = n_rows

    d_sched = np.empty(n_super, np.int32)
    for s in range(n_super):
        d = int(deg[node_order[s * grp]])
        d_sched[s] = max(RU, -(-d // RU) * RU)

    nch = SN // P
    col_start = np.zeros(n_super + 1, np.int64)
    for s in range(n_super):
        col_start[s + 1] = col_start[s] + nch + nch * d_sched[s]
    s_cols = int(col_start[-1])

    pos = invorder
    g_of = pos // grp
    within = pos % grp
    core_of = within // SN
    t_of = within % SN
    chunk_of = t_of // P
    p_of = t_of % P
    rowmap = (core_of * b_rows + g_of * SN + t_of).astype(np.int64)

    idx_all = np.full((N_CORES, P, s_cols), ghost_row, np.int64)
    self_col = col_start[g_of] + chunk_of
    idx_all[core_of, p_of, self_col] = rowmap[np.arange(N_NODES)]

    order = np.argsort(dst, kind="stable")
    src_s = src[order]
    dst_s = dst[order]
    seg_start = np.zeros(N_NODES, np.int64)
    np.cumsum(deg[:-1], out=seg_start[1:])
    rank = np.arange(e2) - seg_start[dst_s]
    n = dst_s
    col = col_start[g_of[n]] + nch + chunk_of[n] * d_sched[g_of[n]] + rank
    idx_all[core_of[n], p_of[n], col] = rowmap[src_s]

    return dict(n_super=n_super, b_rows=b_rows, n_rows=n_rows,
                ghost_row=ghost_row, d_sched=d_sched, col_start=col_start,
                s_cols=s_cols, node_order=node_order, rowmap=rowmap,
                idx_all=idx_all, nch=nch)


# ------------------------------------------------------------- bass program --
def _build_program(pp):
    import concourse.bass as bass
    import concourse.tile as tile
    import concourse.mybir as mybir
    from concourse.bacc import Bacc
    from concourse.masks import make_identity
    from concourse.bass import IndirectOffsetOnAxis

    F32 = mybir.dt.float32
    I32 = mybir.dt.int32
    AX = mybir.AxisListType
    OP = mybir.AluOpType
    AF = mybir.ActivationFunctionType

    n_super, nch = pp["n_super"], pp["nch"]
    d_sched, col_start = pp["d_sched"], pp["col_start"]
    b_rows, n_rows, s_cols = pp["b_rows"], pp["n_rows"], pp["s_cols"]
    d_max = int(max(d_sched))
    H = HEADS
    CH1 = H * C1
    CH2 = H * NCLS
    W1 = CH1 + 2 * H          # 80
    W2 = CH2 + 2 * H          # 72
    KT = F_IN // 128

    def vv(ap, off, dims):
        return bass.AP(ap.tensor, ap.offset + off,
                       [list(ap.ap[0])] + [[s, c] for s, c in dims])

    def dv(t_ap, off, dims):
        return bass.AP(t_ap.tensor, t_ap.offset + off, [[s, c] for s, c in dims])

    nc = Bacc("TRN2", target_bir_lowering=False, debug=False,
              num_devices=N_CORES)

    xT = nc.dram_tensor("xT", [F_IN, b_rows], F32, kind="ExternalInput")
    idx_in = nc.dram_tensor("idx", [P, s_cols], I32, kind="ExternalInput")
    wa1 = nc.dram_tensor("wa1", [F_IN, W1], F32, kind="ExternalInput")
    wa2 = nc.dram_tensor("wa2", [CH1, W2], F32, kind="ExternalInput")
    b1_in = nc.dram_tensor("b1", [1, CH1], F32, kind="ExternalInput")
    b2_in = nc.dram_tensor("b2", [1, NCLS], F32, kind="ExternalInput")
    out = nc.dram_tensor("out", [b_rows, NCLS], F32, kind="ExternalOutput")

    t1 = nc.dram_tensor("t1", [n_rows + 8, W1], F32, kind="Internal",
                        addr_space="Shared")
    t2 = nc.dram_tensor("t2", [n_rows + 8, W2], F32, kind="Internal",
                        addr_space="Shared")
    t1b = nc.dram_tensor("t1b", [b_rows, W1], F32, kind="Internal")
    t2b = nc.dram_tensor("t2b", [b_rows, W2], F32, kind="Internal")
    groups = [list(range(N_CORES))]

    def edge_supertile(s, table, W, C, gjp, gip, prodp, sp, b1_sb, idx_sb,
                       concat):
        nchl = nch
        D = int(d_sched[s])
        c0 = int(col_start[s])
        CHW = H * C
        CD = nchl * D
        gj = gjp.tile([P, nchl * d_max * W1], F32, name="gj", tag="gj")
        gi = gip.tile([P, nchl * W1], F32, name="gi", tag="gi")
        for ch in range(nchl):
            nc.gpsimd.indirect_dma_start(
                out=gi[:, ch * W:(ch + 1) * W], out_offset=None,
                in_=table[:, :],
                in_offset=IndirectOffsetOnAxis(
                    ap=idx_sb[:, c0 + ch:c0 + ch + 1], axis=0))
        for sl in range(CD):
            nc.gpsimd.indirect_dma_start(
                out=vv(gj[:], sl * W, [(1, W)]), out_offset=None,
                in_=table[:, :],
                in_offset=IndirectOffsetOnAxis(
                    ap=idx_sb[:, c0 + nchl + sl:c0 + nchl + sl + 1], axis=0))

        gj_h = vv(gj[:], 0, [(W, CD), (C, H), (1, C)])
        gi_h = vv(gi[:], 0, [(W, nchl), (0, D), (C, H), (1, C)])
        prod = prodp.tile([P, nchl * d_max * CH1], F32, name="prod", tag="prod")
        prod_a = vv(prod[:], 0, [(1, CD * CHW)])
        nc.vector.tensor_tensor(out=prod_a, in0=gj_h, in1=gi_h, op=OP.mult)

        logits = sp.tile([P, nchl * d_max * H], F32, name="logits", tag="logits")
        lg_a = vv(logits[:], 0, [(1, CD * H)])
        nc.vector.tensor_reduce(
            out=lg_a, in_=vv(prod[:], 0, [(CHW, CD), (C, H), (1, C)]),
            axis=AX.X, op=OP.add)

        alpha = sp.tile([P, nchl * d_max * H], F32, name="alpha", tag="alpha")
        al_a = vv(alpha[:], 0, [(1, CD * H)])
        nc.vector.tensor_tensor(
            out=al_a, in0=vv(gj[:], CHW, [(W, CD), (1, H)]),
            in1=vv(gi[:], CHW + H, [(W, nchl), (0, D), (1, H)]), op=OP.add)
        sg = sp.tile([P, nchl * d_max * H], F32, name="sg", tag="sg")
        sg_a = vv(sg[:], 0, [(1, CD * H)])
        nc.scalar.activation(sg_a, lg_a, AF.Sigmoid)
        nc.vector.tensor_tensor(out=al_a, in0=al_a, in1=sg_a, op=OP.mult)
        nc.vector.tensor_scalar(out=sg_a, in0=al_a, scalar1=NEG_SLOPE,
                                scalar2=None, op0=OP.mult)
        nc.vector.tensor_tensor(out=al_a, in0=al_a, in1=sg_a, op=OP.max)

        amax = sp.tile([P, nchl * H], F32, name="amax", tag="amax")
        am_a = vv(amax[:], 0, [(1, nchl * H)])
        nc.vector.tensor_reduce(
            out=am_a, in_=vv(alpha[:], 0, [(D * H, nchl), (1, H), (H, D)]),
            axis=AX.X, op=OP.max)
        nc.vector.tensor_tensor(
            out=al_a, in0=al_a,
            in1=vv(amax[:], 0, [(H, nchl), (0, D), (1, H)]), op=OP.subtract)
        nc.scalar.activation(al_a, al_a, AF.Exp)
        den = sp.tile([P, nchl * H], F32, name="den", tag="den")
        dn_a = vv(den[:], 0, [(1, nchl * H)])
        nc.vector.tensor_reduce(
            out=dn_a, in_=vv(alpha[:], 0, [(D * H, nchl), (1, H), (H, D)]),
            axis=AX.X, op=OP.add)
        nc.vector.tensor_scalar(out=dn_a, in0=dn_a, scalar1=1e-16,
                                scalar2=None, op0=OP.add)
        nc.vector.reciprocal(out=dn_a, in_=dn_a)
        nc.vector.tensor_tensor(
            out=al_a, in0=al_a,
            in1=vv(den[:], 0, [(H, nchl), (0, D), (1, H)]), op=OP.mult)

        nc.vector.tensor_tensor(
            out=vv(prod[:], 0, [(CHW, CD), (C, H), (1, C)]), in0=gj_h,
            in1=vv(alpha[:], 0, [(H, CD), (1, H), (0, C)]), op=OP.mult)
        op_t = gip.tile([P, nchl * CH1], F32, name="opre", tag="opre")
        opv = vv(op_t[:], 0, [(1, nchl * CHW)])
        nc.vector.tensor_reduce(
            out=opv,
            in_=vv(prod[:], 0, [(D * CHW, nchl), (C, H), (1, C), (CHW, D)]),
            axis=AX.X, op=OP.add)
        if not concat:
            return op_t
        nc.vector.tensor_tensor(
            out=opv, in0=opv,
            in1=vv(b1_sb[:], 0, [(0, nchl), (1, CHW)]), op=OP.add)
        zm = gip.tile([P, nchl * CHW], F32, name="zm", tag="zm")
        nc.vector.tensor_scalar(out=zm[:], in0=op_t[:, :nchl * CHW],
                                scalar1=0.0, scalar2=None, op0=OP.min)
        nc.scalar.activation(zm[:], zm[:], AF.Exp)
        nc.vector.tensor_scalar(out=opv, in0=opv, scalar1=0.0, scalar2=None,
                                op0=OP.max)
        nc.vector.tensor_tensor(out=opv, in0=opv, in1=zm[:], op=OP.add)
        nc.vector.tensor_scalar(out=opv, in0=opv, scalar1=-1.0, scalar2=None,
                                op0=OP.add)
        return op_t

    with tile.TileContext(nc) as tc:
        with (
            tc.tile_pool(name="const", bufs=1) as cpool,
            tc.tile_pool(name="xk", bufs=3) as xkpool,
            tc.tile_pool(name="rowout", bufs=3) as rowpool,
            tc.tile_pool(name="gj", bufs=2) as gjpool,
            tc.tile_pool(name="gi", bufs=2) as gipool,
            tc.tile_pool(name="prod", bufs=1) as prodpool,
            tc.tile_pool(name="small", bufs=2) as spool,
            tc.tile_pool(name="tail", bufs=2) as tailpool,
            tc.tile_pool(name="psum", bufs=2, space="PSUM") as pspool,
        ):
            wa1_sb = [cpool.tile([P, W1], F32, name=f"wa1_{k}", tag=f"wa1_{k}")
                      for k in range(KT)]
            for k in range(KT):
                nc.sync.dma_start(out=wa1_sb[k][:],
                                  in_=wa1[k * 128:(k + 1) * 128, :])
            wa2_sb = cpool.tile([CH1, W2], F32, name="wa2", tag="wa2")
            nc.sync.dma_start(out=wa2_sb[:], in_=wa2[:, :])
            ident = cpool.tile([P, P], F32, name="ident", tag="ident")
            make_identity(nc, ident[:])
            b1_sb = cpool.tile([P, CH1], F32, name="b1s", tag="b1s")
            nc.sync.dma_start(out=b1_sb[:1, :], in_=b1_in[:, :])
            nc.gpsimd.partition_broadcast(b1_sb[:], b1_sb[:1, :])
            b2_sb = cpool.tile([P, NCLS], F32, name="b2s", tag="b2s")
            nc.sync.dma_start(out=b2_sb[:1, :], in_=b2_in[:, :])
            nc.gpsimd.partition_broadcast(b2_sb[:], b2_sb[:1, :])
            idx_sb = cpool.tile([P, s_cols], I32, name="idxs", tag="idxs")
            nc.sync.dma_start(out=idx_sb[:], in_=idx_in[:, :])
            g1_sb = cpool.tile([1, W1], F32, name="g1", tag="g1")
            nc.vector.memset(g1_sb[:, :], 0.0)
            nc.vector.memset(g1_sb[:, CH1:CH1 + H], -1e30)
            nc.sync.dma_start(out=t1[n_rows:n_rows + 1, :], in_=g1_sb[:, :])
            g2_sb = cpool.tile([1, W2], F32, name="g2", tag="g2")
            nc.vector.memset(g2_sb[:, :], 0.0)
            nc.vector.memset(g2_sb[:, CH2:CH2 + H], -1e30)
            nc.sync.dma_start(out=t2[n_rows:n_rows + 1, :], in_=g2_sb[:, :])

            for s in range(n_super):
                xk = [xkpool.tile([P, SN], F32, name=f"xk{k}", tag=f"xk{k}")
                      for k in range(KT)]
                for k in range(KT):
                    nc.sync.dma_start(
                        out=xk[k][:],
                        in_=xT[k * 128:(k + 1) * 128, s * SN:(s + 1) * SN])
                row_sb = rowpool.tile([P, nch * W1], F32, name="row1", tag="row1")
                for ch in range(nch):
                    ps = pspool.tile([P, W1], F32, name="psmm", tag="psmm")
                    for k in range(KT):
                        nc.tensor.matmul(
                            out=ps[:], lhsT=xk[k][:, ch * 128:(ch + 1) * 128],
                            rhs=wa1_sb[k][:], start=(k == 0), stop=(k == KT - 1))
                    nc.scalar.copy(out=row_sb[:, ch * W1:(ch + 1) * W1],
                                   in_=ps[:])
                nc.sync.dma_start(
                    out=dv(t1b[:], s * SN * W1,
                           [(W1, P), (P * W1, nch), (1, W1)]),
                    in_=row_sb[:])

            nc.gpsimd.collective_compute(
                "AllGather", OP.bypass, replica_groups=groups,
                ins=[t1b[:, :]], outs=[t1[0:n_rows, :]])

            for s in range(n_super):
                z = edge_supertile(s, t1, W1, C1, gjpool, gipool, prodpool,
                                   spool, b1_sb, idx_sb, concat=True)
                row2 = rowpool.tile([P, nch * W2], F32, name="row2", tag="row2")
                for ch in range(nch):
                    trp = pspool.tile([64, P], F32, name="pstr", tag="pstr")
                    nc.tensor.transpose(
                        out=trp[:], in_=z[:, ch * CH1:(ch + 1) * CH1],
                        identity=ident[:])
                    zt = tailpool.tile([CH1, P], F32, name="zt", tag="zt")
                    nc.scalar.copy(out=zt[:], in_=trp[:])
                    ps2 = pspool.tile([P, W2], F32, name="psm2", tag="psm2")
                    nc.tensor.matmul(out=ps2[:], lhsT=zt[:], rhs=wa2_sb[:],
                                     start=True, stop=True)
                    nc.scalar.copy(out=row2[:, ch * W2:(ch + 1) * W2],
                                   in_=ps2[:])
                nc.sync.dma_start(
                    out=dv(t2b[:], s * SN * W2,
                           [(W2, P), (P * W2, nch), (1, W2)]),
                    in_=row2[:])

            nc.gpsimd.collective_compute(
                "AllGather", OP.bypass, replica_groups=groups,
                ins=[t2b[:, :]], outs=[t2[0:n_rows, :]])

            for s in range(n_super):
                op_t = edge_supertile(s, t2, W2, NCLS, gjpool, gipool,
                                      prodpool, spool, None, idx_sb,
                                      concat=False)
                mp = tailpool.tile([P, nch * NCLS], F32, name="meanp", tag="meanp")
                nc.vector.tensor_reduce(
                    out=mp[:],
                    in_=vv(op_t[:], 0, [(H * NCLS, nch), (1, NCLS), (NCLS, H)]),
                    axis=AX.X, op=OP.add)
                lg2 = tailpool.tile([P, nch * NCLS], F32, name="lg2", tag="lg2")
                nc.vector.tensor_scalar(out=lg2[:], in0=mp[:], scalar1=1.0 / H,
                                        scalar2=None, op0=OP.mult)
                nc.vector.tensor_tensor(
                    out=lg2[:], in0=lg2[:],
                    in1=vv(b2_sb[:], 0, [(0, nch), (1, NCLS)]), op=OP.add)
                mx = spool.tile([P, nch], F32, name="mx", tag="mx")
                nc.vector.tensor_reduce(
                    out=mx[:], in_=vv(lg2[:], 0, [(NCLS, nch), (1, NCLS)]),
                    axis=AX.X, op=OP.max)
                sh = tailpool.tile([P, nch * NCLS], F32, name="sh", tag="sh")
                nc.vector.tensor_tensor(
                    out=sh[:], in0=lg2[:],
                    in1=vv(mx[:], 0, [(1, nch), (0, NCLS)]), op=OP.subtract)
                exl = tailpool.tile([P, nch * NCLS], F32, name="exl", tag="exl")
                nc.scalar.activation(exl[:], sh[:], AF.Exp)
                sm = spool.tile([P, nch], F32, name="sm", tag="sm")
                nc.vector.tensor_reduce(
                    out=sm[:], in_=vv(exl[:], 0, [(NCLS, nch), (1, NCLS)]),
                    axis=AX.X, op=OP.add)
                lsm = spool.tile([P, nch], F32, name="lsm", tag="lsm")
                nc.scalar.activation(lsm[:], sm[:], AF.Ln)
                nc.vector.tensor_tensor(out=lsm[:], in0=lsm[:], in1=mx[:],
                                        op=OP.add)
                of = tailpool.tile([P, nch * NCLS], F32, name="of", tag="of")
                nc.vector.tensor_tensor(
                    out=of[:], in0=lg2[:],
                    in1=vv(lsm[:], 0, [(1, nch), (0, NCLS)]), op=OP.subtract)
                nc.sync.dma_start(
                    out=dv(out[:], s * SN * NCLS,
                           [(NCLS, P), (P * NCLS, nch), (1, NCLS)]),
                    in_=of[:])

    nc.compile()
    return nc


# -------------------------------------------------------------------- entry --
LAST_RESULT = None   # BassKernelResults of the most recent run (for profiling)


def _install_trace_hook():
    """Best-effort NTFF profile hook for axon (used only with GAT_TRACE=1)."""
    import types
    try:
        import antenv
        if "antenv.axon_hooks" not in sys.modules:
            mod = types.ModuleType("antenv.axon_hooks")
            holder = {}
            mod.set_axon_ntff_profile_hook = lambda h: holder.__setitem__("h", h)
            mod.get_axon_ntff_profile_hook = lambda: holder.get("h")
            sys.modules["antenv.axon_hooks"] = mod
            antenv.axon_hooks = mod
        if "/root/.axon_site" not in sys.path:
            sys.path.insert(0, "/root/.axon_site")
        from trn_agent_boot.trn_boot import _ntff_profile_via_ctypes
        hook = _ntff_profile_via_ctypes("/opt/axon/libaxon_pjrt.so")
        sys.modules["antenv.axon_hooks"].set_axon_ntff_profile_hook(hook)
        import concourse.bass_utils as bu
        bu.upload_artifacts = lambda tmpdir: f"file://{tmpdir}"
        return True
    except Exception:
        return False


def kernel(**inputs):
    import os
    from concourse import bass_utils

    trace = bool(os.environ.get("GAT_TRACE"))
    if trace:
        trace = _install_trace_hook()

    pp = _preprocess(np.asarray(inputs["edge_index"]))
    key = (pp["n_super"], tuple(int(d) for d in pp["d_sched"]),
           pp["b_rows"], pp["s_cols"])
    nc = _nc_cache.get(key)
    if nc is None:
        nc = _build_program(pp)
        _nc_cache[key] = nc

    x = np.asarray(inputs["x"], np.float32)
    W1m = np.asarray(inputs["W1"], np.float32)
    W2m = np.asarray(inputs["W2"], np.float32)
    A_l1 = _head_expand(inputs["att_l1"], HEADS, C1)
    A_r1 = _head_expand(inputs["att_r1"], HEADS, C1)
    A_l2 = _head_expand(inputs["att_l2"], HEADS, NCLS)
    A_r2 = _head_expand(inputs["att_r2"], HEADS, NCLS)
    wa1 = np.ascontiguousarray(
        np.concatenate([W1m, W1m @ A_l1, W1m @ A_r1], 1).astype(np.float32))
    wa2 = np.ascontiguousarray(
        np.concatenate([W2m, W2m @ A_l2, W2m @ A_r2], 1).astype(np.float32))
    b1 = np.asarray(inputs["b1"], np.float32).reshape(1, -1)
    b2 = np.asarray(inputs["b2"], np.float32).reshape(1, -1)

    grp = N_CORES * SN
    in_maps = []
    for c in range(N_CORES):
        xb = np.zeros((pp["b_rows"], F_IN), np.float32)
        for s in range(pp["n_super"]):
            nodes = pp["node_order"][s * grp + c * SN:s * grp + (c + 1) * SN]
            xb[s * SN:s * SN + len(nodes)] = x[nodes]
        in_maps.append({
            "xT": np.ascontiguousarray(xb.T),
            "idx": np.ascontiguousarray(pp["idx_all"][c].astype(np.int32)),
            "wa1": wa1, "wa2": wa2, "b1": b1, "b2": b2,
        })

    res = bass_utils.run_bass_kernel_spmd(
        nc, in_maps, core_ids=list(range(N_CORES)), trace=trace)
    global LAST_RESULT
    LAST_RESULT = res
    outs = np.stack([res.results[c]["out"] for c in range(N_CORES)])
    flat = outs.reshape(N_CORES * pp["b_rows"], NCLS)
    final = np.ascontiguousarray(flat[pp["rowmap"][:N_NODES]]).astype(np.float32)
    return final, np.float32(0.0)
